# revision 8
# baseline (speedup 1.0000x reference)
"""Complex-valued causal attention head on 8 Trainium2 NeuronCores.

Math (per batch element, fp32 reference):
    q/k/v = complex_linear(x, W*)          # contract C=1024 -> H=64
    wr + i*wi = q @ conj(k)^T              # contract H
    mag = sqrt(wr^2 + wi^2 + 1e-4) / sqrt(H)
    wei = softmax(causal_mask(mag))
    out = wei @ v   (real and imag parts separately)

Sharding: data-parallel over batch B=8 -> one batch element per core, weights
replicated, no collectives. Host-side prep is layout-only + 16-bit cast.

Per-core dataflow (T=2048, C=1024, H=64):
  - All matmul operands are 16-bit (PSUM accumulates fp32): fp32r moving
    operands stream at ~0.83 ns/row on the PE, 16-bit at ~0.42 ns/row, so
    this halves the tensor-engine critical path. Dtype split by range/
    precision: fp16 for x/W/k/q/scores^2 (small-range values where bf16's
    8-bit mantissa costs accuracy through the exp), bf16 for p/v/out^T
    (p reaches ~5e6 and sum(p*v) ~2e7 -> fp16 would overflow).
  - Squares are pre-scaled by 1/4 inside the exits (s' = |w|^2/16) for fp16
    headroom; the ln/exp chain absorbs the 1/16 in its bias.
  - Complex projections: pre-stacked weight pairs [Wr|Wi] / [-Wi|Wr] let PSUM
    accumulation do all the complex combines; outputs come out H-stacked and
    transposed: K+=[kr;ki], Q+=[qr;qi], V+=[vr;vi], each [128, T-chunk].
    Q2=[-qi;qr] is derived from Q+ with one signed-permutation matmul.
  - x and weights are DMA'd in fine-grained tiles (x in 4 cc-pair tiles per
    chunk, weights split per projection) so the first projection matmuls
    start as soon as the first slices land instead of after the full load.
  - Scores computed TRANSPOSED [tk, tq]: psRe = K+[:,tk]^T @ Q+,
    psIm = K+[:,tk]^T @ Q2 (sign dies in squaring); probabilities come out
    as the p^T operand that the PV matmul and ones-matmul row-sum need.
  - mag^2 exits from PSUM: re^2 via ACT Square(scale=1/4) or DVE
    tensor_scalar_mul+mult (split to balance engines; PSUM's single read
    port per engine forbids tensor_tensor(ps,ps)); im^2 via DVE. GPSIMD
    adds QUAD-wide; ACT runs ln/exp/exp OCT-wide (4 tk-blocks batched):
        p = exp(exp(0.5*ln(s' + eps/16) + ln(1/2)))
    (square/ln/exp share one ACT table set -> no table reloads).
  - Causal mask on diagonal blocks via in-place gpsimd affine_select (p:=0).
  - Row sums via ones-matmul on PE (M=1); PV accumulates out^T [h2, tq].
    The sums/PV matmuls for an OCT group are emitted LAG groups behind the
    score/softmax emissions, and the tail groups of each chunk are carried
    over past the next chunk's projection matmuls, so the PE never waits
    on the elementwise chain.
  - out^T is PE-transposed back to natural [t, h2]; the row 1/sum scaling
    rides the PSUM->SBUF copy via tensor_scalar_mul; DMA out.
"""

import numpy as np

B, T, C, H = 8, 2048, 1024, 64
H2 = 2 * H            # stacked real|imag head dim = 128
P = 128               # partitions
NCHUNK = 4            # T / 512
CH = T // NCHUNK      # 512 tq columns per chunk
TB = T // P           # 16 tk blocks
EPS = 1e-4
QUAD = 2 * CH         # gpsimd add width (2 tk-blocks)
OCT = 4 * CH          # ACT chain width (4 tk-blocks)
LAG = 2               # OCT groups of score->softmax in flight before PV
ACT_EXIT_NUM, ACT_EXIT_DEN = 3, 8   # fraction of ALL exits taken by ACT

_BUILT = None


def _build(loop_n=None):
    import contextlib

    import concourse.bass as bass
    import concourse.mybir as mybir
    import concourse.tile as tile

    f32 = mybir.dt.float32
    f16 = mybir.dt.float16
    b16 = mybir.dt.bfloat16
    AF = mybir.ActivationFunctionType
    ALU = mybir.AluOpType

    nc = bass.Bass(trn_type="TRN2")

    # x pre-transposed AND partition-major: [chunk, part(4), p, 2, t] so each
    # partition reads one contiguous run per sub-tile DMA
    xr_d = nc.dram_tensor("xT_real", [NCHUNK, 4, P, 2, CH], f16, kind="ExternalInput").ap()
    xi_d = nc.dram_tensor("xT_imag", [NCHUNK, 4, P, 2, CH], f16, kind="ExternalInput").ap()
    # weight stacks [P, pair(3: K,Q,V), 2, CC, H2]; pair p: [:,p,0]=S1, [:,p,1]=S2
    wst_d = nc.dram_tensor("wstacks", [P, 3, 2, C // P, H2], f16, kind="ExternalInput").ap()
    # consts16: [:, :128]=S_T perm fp16 ; consts_b: ident bf16 + ones bf16
    cper_d = nc.dram_tensor("consts_perm", [P, P], f16, kind="ExternalInput").ap()
    cidn_d = nc.dram_tensor("consts_ident", [P, P + 1], b16, kind="ExternalInput").ap()

    # packed output [chunk, p, tb, h2]; host unpacks to (T, H) r/i halves
    out_d = nc.dram_tensor("out_pk", [NCHUNK, P, 4, H2], f32, kind="ExternalOutput").ap()

    CC = C // P  # 8 contraction chunks

    with tile.TileContext(nc) as tc:
        ctx = contextlib.ExitStack()
        with ctx:
            if loop_n is not None:
                ctx.enter_context(tc.For_i(0, loop_n, 1))
            singles = ctx.enter_context(tc.tile_pool(name="singles", bufs=1))
            xt_p = ctx.enter_context(tc.tile_pool(name="xt", bufs=2))
            qc_p = ctx.enter_context(tc.tile_pool(name="qc", bufs=2))
            elw_p = ctx.enter_context(tc.tile_pool(name="elw", bufs=2))
            im_p = ctx.enter_context(tc.tile_pool(name="imp", bufs=4))
            p_p = ctx.enter_context(tc.tile_pool(name="pp", bufs=LAG + 2))
            fin_p = ctx.enter_context(tc.tile_pool(name="fin", bufs=2))

            # PSUM budget is 8 banks (2KB/partition each), reserved statically:
            # projps 1 + scps 3 + accps(out+sums) 2 + finps 2 = 8
            projps = ctx.enter_context(tc.tile_pool(name="projps", bufs=1, space="PSUM"))
            scps = ctx.enter_context(tc.tile_pool(name="scps", bufs=3, space="PSUM"))
            accps = ctx.enter_context(tc.tile_pool(name="accps", bufs=1, space="PSUM"))
            finps = ctx.enter_context(tc.tile_pool(name="finps", bufs=1, space="PSUM"))

            # ---- weights / constants (fine-grained so proj-K starts early) ----
            wk = singles.tile([P, 2, CC, H2], f16)
            wq = singles.tile([P, 2, CC, H2], f16)
            wv = singles.tile([P, 2, CC, H2], f16)
            nc.sync.dma_start(wk, wst_d[:, 0])

            s_perm = singles.tile([P, P], f16)
            nc.scalar.dma_start(s_perm, cper_d)
            cidn = singles.tile([P, P + 1], b16)
            nc.scalar.dma_start(cidn, cidn_d)
            ident = cidn[:, 0:P]
            ones_col = cidn[:, P:P + 1]

            one1_f32 = singles.tile([1, 1], f32)
            nc.vector.memset(one1_f32, 1.0)
            bias_eps = singles.tile([P, 1], f32)
            nc.vector.memset(bias_eps, EPS / 16.0)
            bias_lnc = singles.tile([P, 1], f32)
            nc.vector.memset(bias_lnc, float(np.log(0.5)))
            bias_zero = singles.tile([P, 1], f32)
            nc.vector.memset(bias_zero, 0.0)

            # ---- persistent per-batch buffers ----
            k_all = singles.tile([P, T], f16)        # K+ = [kr^T; ki^T]
            v_nat = singles.tile([P, TB, H2], b16)   # V natural [t, h2] blocks

            exit_ctr = [0]   # global counter for ACT/DVE re^2 exit split
            pending = []     # deferred (stage_b, args) across chunks
            prev_fin = [None]

            def load_chunk(j):
                # chunk 0 is latency-critical: split xr/xi across the two
                # HWDGE queues. Later chunks are prefetch -> keep them off
                # the scalar queue so its engine (ACT) isn't burdened with
                # descriptor-generation instructions.
                xi_eng = nc.scalar if j == 0 else nc.sync
                xr_parts, xi_parts = [], []
                for part in range(4):
                    xr_t = xt_p.tile([P, 2, CH], f16, tag=f"xtr{part}")
                    nc.sync.dma_start(xr_t, xr_d[j, part])
                    xr_parts.append(xr_t)
                for part in range(4):
                    xi_t = xt_p.tile([P, 2, CH], f16, tag=f"xti{part}")
                    xi_eng.dma_start(xi_t, xi_d[j, part])
                    xi_parts.append(xi_t)
                return xr_parts, xi_parts

            # chunk-0 loads + remaining weights (emitted after, so wk/x win
            # the queues and proj-K starts as early as possible)
            xparts = load_chunk(0)
            nc.sync.dma_start(wq, wst_d[:, 1])
            nc.sync.dma_start(wv, wst_d[:, 2])

            for j in range(NCHUNK):
                xr_parts, xi_parts = xparts

                # ---------- projections (PSUM-accumulated complex) ----------
                def proj(w2):
                    ps = projps.tile([P, CH], f32, tag="projps")
                    for cc in range(CC):
                        nc.tensor.matmul(ps, w2[:, 0, cc],
                                         xr_parts[cc // 2][:, cc % 2],
                                         start=(cc == 0), stop=False)
                    for cc in range(CC):
                        nc.tensor.matmul(ps, w2[:, 1, cc],
                                         xi_parts[cc // 2][:, cc % 2],
                                         start=False, stop=(cc == CC - 1))
                    return ps

                c0, c1 = j * CH, (j + 1) * CH
                ps_k = proj(wk)
                nc.vector.tensor_copy(k_all[:, c0:c1], ps_k)

                ps_q = proj(wq)
                q_c = qc_p.tile([P, CH], f16, tag="qc")
                nc.scalar.copy(q_c, ps_q)

                # Q2 = [-qi; qr] = S @ Q+ via one signed-permutation matmul
                ps_q2 = scps.tile([P, CH], f32, tag="sc")
                nc.tensor.matmul(ps_q2, s_perm, q_c, start=True, stop=True)
                q2_c = qc_p.tile([P, CH], f16, tag="q2c")
                nc.vector.tensor_copy(q2_c, ps_q2)

                ps_v = proj(wv)
                vt_c = qc_p.tile([P, CH], b16, tag="vtc")
                nc.vector.tensor_copy(vt_c, ps_v)
                # V+ [h2, t] -> natural [t, h2] blocks
                ps_vn = finps.tile([P, 4, H2], b16, tag="vno")
                for t4 in range(4):
                    nc.tensor.transpose(
                        ps_vn[:, t4], vt_c[:, t4 * P:(t4 + 1) * P], ident)
                nc.vector.tensor_copy(v_nat[:, j * 4:(j + 1) * 4], ps_vn)

                # prefetch next chunk's x behind this chunk's loads
                if j + 1 < NCHUNK:
                    xparts = load_chunk(j + 1)

                # deferred PV/sums of the previous chunk overlap proj above;
                # then the previous chunk can finalize
                while pending:
                    fn, args = pending.pop(0)
                    fn(*args)
                if prev_fin[0] is not None:
                    prev_fin[0]()
                    prev_fin[0] = None

                # ---------- scores / softmax / PV over tk blocks ----------
                ps_out = accps.tile([P, CH], f32, tag="outps")
                ps_sums = accps.tile([1, CH], f32, tag="sumps")
                nblk = 4 * (j + 1)
                # OCT groups; the LAST chunk's tail is split into QUADs to
                # shorten the exposed end-of-kernel latency chain
                if j == NCHUNK - 1:
                    groups = [list(range(g, g + 4)) for g in range(0, nblk - 4, 4)]
                    groups += [[nblk - 4, nblk - 3], [nblk - 2, nblk - 1]]
                else:
                    groups = [list(range(g, g + 4)) for g in range(0, nblk, 4)]

                def stage_a(blocks, q_c=q_c, q2_c=q2_c, j=j):
                    """scores + exits + squares + adds + ln/exp/exp for a
                    group of tk blocks. Returns the p tile."""
                    nb = len(blocks)
                    w = nb * CH
                    sq1 = elw_p.tile([P, w], f16, tag=f"sq1w{nb}")
                    sq2 = elw_p.tile([P, w], f16, tag=f"sq2w{nb}")
                    s_t = elw_p.tile([P, w], f16, tag=f"stw{nb}")
                    for b4, i in enumerate(blocks):
                        kT = k_all[:, i * P:(i + 1) * P]
                        ps_re = scps.tile([P, CH], f32, tag="sc")
                        nc.tensor.matmul(ps_re, kT, q_c, start=True, stop=True)
                        ps_im = scps.tile([P, CH], f32, tag="sc")
                        nc.tensor.matmul(ps_im, kT, q2_c, start=True, stop=True)
                        # each exit: ACT fused square (reads PSUM) or DVE
                        # scale-copy+mult, split to balance engine load
                        for ps_s, sq in ((ps_re, sq1), (ps_im, sq2)):
                            cs = slice(b4 * CH, (b4 + 1) * CH)
                            if (exit_ctr[0] % ACT_EXIT_DEN) < ACT_EXIT_NUM:
                                nc.scalar.activation(
                                    sq[:, cs], ps_s, AF.Square,
                                    bias=bias_zero, scale=0.25)
                            else:
                                e_s = im_p.tile([P, CH], f16, tag="es")
                                nc.vector.tensor_scalar_mul(e_s, ps_s, 0.25)
                                nc.vector.tensor_tensor(
                                    sq[:, cs], e_s, e_s, ALU.mult)
                            exit_ctr[0] += 1
                        if b4 % 2 == 1:  # QUAD-wide adds as halves complete
                            qs = slice((b4 - 1) * CH, (b4 + 1) * CH)
                            nc.gpsimd.tensor_tensor(
                                s_t[:, qs], sq1[:, qs], sq2[:, qs], ALU.add)

                    m_t = elw_p.tile([P, w], f32, tag=f"mtw{nb}")
                    # ln and first exp run in place over m_t (group-wide)
                    nc.scalar.activation(m_t, s_t, AF.Ln,
                                         bias=bias_eps, scale=1.0)
                    nc.scalar.activation(m_t, m_t, AF.Exp,
                                         bias=bias_lnc, scale=0.5)
                    p_t = p_p.tile([P, w], b16, tag=f"ptw{nb}")
                    nc.scalar.activation(p_t, m_t, AF.Exp,
                                         bias=bias_zero, scale=1.0)
                    return p_t

                def stage_b(blocks, p_t, first, last,
                            ps_out=ps_out, ps_sums=ps_sums, j=j):
                    """causal mask + row-sum + PV accumulation matmuls.
                    Masking here (LAG groups behind stage_a) keeps the
                    gpsimd FIFO from head-of-line blocking the adds."""
                    for b4, i in enumerate(blocks):
                        p_blk = p_t[:, b4 * CH:(b4 + 1) * CH]
                        if i >= 4 * j:  # diagonal: zero where tq < tk
                            nc.gpsimd.affine_select(
                                out=p_blk, in_=p_blk,
                                compare_op=ALU.is_ge,
                                fill=0.0,
                                base=j * CH - i * P,
                                pattern=[[1, CH]],
                                channel_multiplier=-1)
                    for b4, i in enumerate(blocks):
                        p_blk = p_t[:, b4 * CH:(b4 + 1) * CH]
                        st = first and b4 == 0
                        sp = last and b4 == len(blocks) - 1
                        nc.tensor.matmul(ps_sums, ones_col, p_blk,
                                         start=st, stop=sp)
                        nc.tensor.matmul(ps_out, v_nat[:, i], p_blk,
                                         start=st, stop=sp)

                for gi, blocks in enumerate(groups):
                    p_t = stage_a(blocks)
                    pending.append(
                        (stage_b,
                         (blocks, p_t, gi == 0, gi == len(groups) - 1)))
                    if len(pending) > LAG:
                        fn, args = pending.pop(0)
                        fn(*args)

                def finalize(j=j, ps_out=ps_out, ps_sums=ps_sums):
                    outT = fin_p.tile([P, CH], b16, tag="outT")
                    nc.vector.tensor_copy(outT, ps_out)
                    sums_sb = fin_p.tile([1, CH], f32, tag="sums")
                    nc.vector.tensor_copy(sums_sb, ps_sums)

                    ps_on = finps.tile([P, 4, H2], b16, tag="vno")
                    for t4 in range(4):
                        nc.tensor.transpose(
                            ps_on[:, t4], outT[:, t4 * P:(t4 + 1) * P], ident)
                    ps_rs = finps.tile([P, 4], f32, tag="rsps")
                    for t4 in range(4):
                        nc.tensor.matmul(ps_rs[:, t4:t4 + 1],
                                         sums_sb[0:1, t4 * P:(t4 + 1) * P],
                                         one1_f32, start=True, stop=True)
                    recip = fin_p.tile([P, 4], f32, tag="recip")
                    nc.vector.reciprocal(recip, ps_rs)

                    onat = fin_p.tile([P, 4, H2], f32, tag="onat")
                    for t4 in range(4):
                        nc.vector.tensor_scalar_mul(
                            onat[:, t4], ps_on[:, t4], recip[:, t4:t4 + 1])
                    nc.sync.dma_start(out_d[j], onat)

                prev_fin[0] = finalize

            # drain the last chunk
            while pending:
                fn, args = pending.pop(0)
                fn(*args)
            prev_fin[0]()

    _split_multiwaits(nc)
    return nc


def _split_multiwaits(nc):
    """This toolchain's walrus accepts at most ONE sync-wait per instruction;
    Tile's sem-assignment can attach several. Hoist all-but-one wait onto
    standalone InstEventSemaphore carriers (what bass's wait_ge emits)."""
    import concourse.mybir as mybir

    n_split = 0
    for f in nc.m.functions:
        for bb in f.blocks:
            out = []
            for inst in bb.instructions:
                si = inst.sync_info
                if si is not None and si.on_wait and len(si.on_wait) > 1:
                    waits = list(si.on_wait)
                    for w in waits[:-1]:
                        carrier = mybir.InstEventSemaphore(
                            name=f"{inst.name}_wsplit{n_split}", ins=[], outs=[])
                        carrier.engine = inst.engine
                        carrier.sync_info = mybir.SyncInfo(
                            on_wait=[w], on_update=[])
                        out.append(carrier)
                        n_split += 1
                    inst.sync_info = mybir.SyncInfo(
                        on_wait=[waits[-1]], on_update=list(si.on_update))
                out.append(inst)
            bb.instructions = out
    return n_split


def _host_prep(Wk_r, Wk_i, Wq_r, Wq_i, Wv_r, Wv_i):
    import ml_dtypes

    f16 = np.float16
    b16 = ml_dtypes.bfloat16

    def s1(wr, wi):
        return np.concatenate([wr, wi], axis=1)

    def s2(wr, wi):
        return np.concatenate([-wi, wr], axis=1)

    # [pair(3), 2, C, H2] -> partition-major [P, 3, 2, CC, H2]
    wst = np.stack([
        [s1(Wk_r, Wk_i), s2(Wk_r, Wk_i)],
        [s1(Wq_r, Wq_i), s2(Wq_r, Wq_i)],
        [s1(Wv_r, Wv_i), s2(Wv_r, Wv_i)],
    ]).astype(f16)
    wst = np.ascontiguousarray(
        wst.reshape(3, 2, C // P, P, H2).transpose(3, 0, 1, 2, 4))
    # S with S @ [qr; qi] = [-qi; qr]; matmul computes lhsT.T @ rhs so pass S^T
    s_mat = np.zeros((P, P), np.float32)
    for i in range(H):
        s_mat[i, H + i] = -1.0
        s_mat[H + i, i] = 1.0
    cper = np.ascontiguousarray(s_mat.T).astype(f16)
    cidn = np.ascontiguousarray(np.concatenate(
        [np.eye(P, dtype=np.float32), np.ones((P, 1), np.float32)],
        axis=1)).astype(b16)
    return wst, cper, cidn


def kernel(x_real, x_imag, Wk_r, Wk_i, Wq_r, Wq_i, Wv_r, Wv_i, _trace=False):
    global _BUILT
    from concourse.bass_utils import run_bass_kernel_spmd

    if _BUILT is None:
        _BUILT = _build()
    nc = _BUILT

    wst, cper, cidn = _host_prep(
        np.asarray(Wk_r), np.asarray(Wk_i), np.asarray(Wq_r),
        np.asarray(Wq_i), np.asarray(Wv_r), np.asarray(Wv_i))
    x_real = np.asarray(x_real, dtype=np.float32)
    x_imag = np.asarray(x_imag, dtype=np.float32)

    def xprep(xb):
        # (T, C) -> xT (C, T) -> [chunk, part, p, 2, t] partition-major, fp16
        return np.ascontiguousarray(
            xb.T.reshape(4, 2, P, NCHUNK, CH).transpose(3, 0, 2, 1, 4)
            .astype(np.float16))

    in_maps = [
        {
            "xT_real": xprep(x_real[b]),
            "xT_imag": xprep(x_imag[b]),
            "wstacks": wst,
            "consts_perm": cper,
            "consts_ident": cidn,
        }
        for b in range(B)
    ]
    res = run_bass_kernel_spmd(nc, in_maps, core_ids=list(range(B)),
                               trace=_trace)
    def unpack(pk):
        # [chunk, p, tb, h2] -> (T, H2)
        full = pk.transpose(0, 2, 1, 3).reshape(T, H2)
        return full[:, 0:H], full[:, H:H2]

    outs = [unpack(res.results[b]["out_pk"]) for b in range(B)]
    out_r = np.ascontiguousarray(np.stack([o[0] for o in outs]))
    out_i = np.ascontiguousarray(np.stack([o[1] for o in outs]))
    if _trace:
        kernel._last_results = res
    return out_r, out_i


# revision 10
# speedup vs baseline: 1.0473x; 1.0473x over previous
"""Complex-valued causal attention head on 8 Trainium2 NeuronCores.

Math (per batch element, fp32 reference):
    q/k/v = complex_linear(x, W*)          # contract C=1024 -> H=64
    wr + i*wi = q @ conj(k)^T              # contract H
    mag = sqrt(wr^2 + wi^2 + 1e-4) / sqrt(H)
    wei = softmax(causal_mask(mag))
    out = wei @ v   (real and imag parts separately)

Sharding: data-parallel over batch B=8 -> one batch element per core, weights
replicated, no collectives. Host-side prep is layout-only + 16-bit cast.

Per-core dataflow (T=2048, C=1024, H=64):
  - All matmul operands are 16-bit (PSUM accumulates fp32): fp32r moving
    operands stream at ~0.83 ns/row on the PE, 16-bit at ~0.42 ns/row, so
    this halves the tensor-engine critical path. Dtype split by range/
    precision: fp16 for x/W/k/q/scores^2 (small-range values where bf16's
    8-bit mantissa costs accuracy through the exp), bf16 for p/v/out^T
    (p reaches ~5e6 and sum(p*v) ~2e7 -> fp16 would overflow).
  - Squares are pre-scaled by 1/4 inside the exits (s' = |w|^2/16) for fp16
    headroom; the ln/exp chain absorbs the 1/16 in its bias.
  - Complex projections: pre-stacked weight pairs [Wr|Wi] / [-Wi|Wr] let PSUM
    accumulation do all the complex combines; outputs come out H-stacked and
    transposed: K+=[kr;ki], Q+=[qr;qi], V+=[vr;vi], each [128, T-chunk].
    Q2=[-qi;qr] is derived from Q+ with one signed-permutation matmul.
  - x and weights are DMA'd in fine-grained tiles (x in 4 cc-pair tiles per
    chunk, weights split per projection) so the first projection matmuls
    start as soon as the first slices land instead of after the full load.
  - Scores computed TRANSPOSED [tk, tq]: psRe = K+[:,tk]^T @ Q+,
    psIm = K+[:,tk]^T @ Q2 (sign dies in squaring); probabilities come out
    as the p^T operand that the PV matmul and ones-matmul row-sum need.
  - mag^2 exits from PSUM: re^2 via ACT Square(scale=1/4) or DVE
    tensor_scalar_mul+mult (split to balance engines; PSUM's single read
    port per engine forbids tensor_tensor(ps,ps)); im^2 via DVE. GPSIMD
    adds QUAD-wide; ACT runs ln/exp/exp OCT-wide (4 tk-blocks batched):
        p = exp(exp(0.5*ln(s' + eps/16) + ln(1/2)))
    (square/ln/exp share one ACT table set -> no table reloads).
  - Causal mask on diagonal blocks via in-place gpsimd affine_select (p:=0).
  - Row sums via ones-matmul on PE (M=1); PV accumulates out^T [h2, tq].
    The sums/PV matmuls for an OCT group are emitted LAG groups behind the
    score/softmax emissions, and the tail groups of each chunk are carried
    over past the next chunk's projection matmuls, so the PE never waits
    on the elementwise chain.
  - out^T is PE-transposed back to natural [t, h2]; the row 1/sum scaling
    rides the PSUM->SBUF copy via tensor_scalar_mul; DMA out.
"""

import numpy as np

B, T, C, H = 8, 2048, 1024, 64
H2 = 2 * H            # stacked real|imag head dim = 128
P = 128               # partitions
NCHUNK = 4            # T / 512
CH = T // NCHUNK      # 512 tq columns per chunk
TB = T // P           # 16 tk blocks
EPS = 1e-4
QUAD = 2 * CH         # gpsimd add width (2 tk-blocks)
OCT = 4 * CH          # ACT chain width (4 tk-blocks)
LAG = 2               # OCT groups of score->softmax in flight before PV
ACT_EXIT_NUM, ACT_EXIT_DEN = 3, 8   # fraction of ALL exits taken by ACT

_BUILT = None


def _build(loop_n=None):
    import contextlib

    import concourse.bass as bass
    import concourse.mybir as mybir
    import concourse.tile as tile

    f32 = mybir.dt.float32
    f16 = mybir.dt.float16
    b16 = mybir.dt.bfloat16
    AF = mybir.ActivationFunctionType
    ALU = mybir.AluOpType

    nc = bass.Bass(trn_type="TRN2")

    # x pre-transposed AND partition-major: [chunk, part(4), p, 2, t] so each
    # partition reads one contiguous run per sub-tile DMA
    xr_d = nc.dram_tensor("xT_real", [NCHUNK, 4, P, 2, CH], f16, kind="ExternalInput").ap()
    xi_d = nc.dram_tensor("xT_imag", [NCHUNK, 4, P, 2, CH], f16, kind="ExternalInput").ap()
    # weight stacks [P, pair(3: K,Q,V), 2, CC, H2]; pair p: [:,p,0]=S1, [:,p,1]=S2
    wst_d = nc.dram_tensor("wstacks", [P, 3, 2, C // P, H2], f16, kind="ExternalInput").ap()
    # consts16: [:, :128]=S_T perm fp16 ; consts_b: ident bf16 + ones bf16
    cper_d = nc.dram_tensor("consts_perm", [P, P], f16, kind="ExternalInput").ap()
    cidn_d = nc.dram_tensor("consts_ident", [P, P + 1], b16, kind="ExternalInput").ap()

    # packed output [chunk, p, tb, h2]; host unpacks to (T, H) r/i halves
    out_d = nc.dram_tensor("out_pk", [NCHUNK, P, 4, H2], f32, kind="ExternalOutput").ap()

    CC = C // P  # 8 contraction chunks

    with tile.TileContext(nc) as tc:
        ctx = contextlib.ExitStack()
        with ctx:
            if loop_n is not None:
                ctx.enter_context(tc.For_i(0, loop_n, 1))
            singles = ctx.enter_context(tc.tile_pool(name="singles", bufs=1))
            xt_p = ctx.enter_context(tc.tile_pool(name="xt", bufs=2))
            qc_p = ctx.enter_context(tc.tile_pool(name="qc", bufs=2))
            elw_p = ctx.enter_context(tc.tile_pool(name="elw", bufs=3))
            mt_p = ctx.enter_context(tc.tile_pool(name="mtp", bufs=2))
            im_p = ctx.enter_context(tc.tile_pool(name="imp", bufs=6))
            p_p = ctx.enter_context(tc.tile_pool(name="pp", bufs=LAG + 2))
            fin_p = ctx.enter_context(tc.tile_pool(name="fin", bufs=2))

            # PSUM budget is 8 banks (2KB/partition each), reserved statically:
            # mixps 5 + accps(out+sums) 2 + finps 1 = 8. One shared 5-deep
            # [P, CH] f32 ring serves projection accumulators, score tiles
            # and the tiny row-sum-transpose output: proj psum is idle during
            # the score phase and vice versa, and the deep ring lets the
            # PE run several score matmuls ahead of the exits.
            mixps = ctx.enter_context(tc.tile_pool(name="mixps", bufs=5, space="PSUM"))
            accps = ctx.enter_context(tc.tile_pool(name="accps", bufs=1, space="PSUM"))
            finps = ctx.enter_context(tc.tile_pool(name="finps", bufs=1, space="PSUM"))

            # ---- weights / constants (fine-grained so proj-K starts early) ----
            wk = singles.tile([P, 2, CC, H2], f16)
            wq = singles.tile([P, 2, CC, H2], f16)
            wv = singles.tile([P, 2, CC, H2], f16)
            nc.sync.dma_start(wk, wst_d[:, 0])

            s_perm = singles.tile([P, P], f16)
            nc.scalar.dma_start(s_perm, cper_d)
            cidn = singles.tile([P, P + 1], b16)
            nc.scalar.dma_start(cidn, cidn_d)
            ident = cidn[:, 0:P]
            ones_col = cidn[:, P:P + 1]

            one1_f32 = singles.tile([1, 1], f32)
            nc.vector.memset(one1_f32, 1.0)
            bias_eps = singles.tile([P, 1], f32)
            nc.vector.memset(bias_eps, EPS / 16.0)
            bias_lnc = singles.tile([P, 1], f32)
            nc.vector.memset(bias_lnc, float(np.log(0.5)))
            bias_zero = singles.tile([P, 1], f32)
            nc.vector.memset(bias_zero, 0.0)

            # ---- persistent per-batch buffers ----
            k_all = singles.tile([P, T], f16)        # K+ = [kr^T; ki^T]
            v_nat = singles.tile([P, TB, H2], b16)   # V natural [t, h2] blocks

            exit_ctr = [0]   # global counter for ACT/DVE re^2 exit split
            pending = []     # deferred (stage_b, args) across chunks
            prev_fin = [None]

            def load_chunk(j):
                # chunk 0 is latency-critical: split xr/xi across the two
                # HWDGE queues. Later chunks are prefetch -> keep them off
                # the scalar queue so its engine (ACT) isn't burdened with
                # descriptor-generation instructions.
                xi_eng = nc.scalar if j == 0 else nc.sync
                xr_parts, xi_parts = [], []
                for part in range(4):
                    xr_t = xt_p.tile([P, 2, CH], f16, tag=f"xtr{part}")
                    nc.sync.dma_start(xr_t, xr_d[j, part])
                    xr_parts.append(xr_t)
                for part in range(4):
                    xi_t = xt_p.tile([P, 2, CH], f16, tag=f"xti{part}")
                    xi_eng.dma_start(xi_t, xi_d[j, part])
                    xi_parts.append(xi_t)
                return xr_parts, xi_parts

            # chunk-0 loads + remaining weights (emitted after, so wk/x win
            # the queues and proj-K starts as early as possible)
            xparts = load_chunk(0)
            nc.sync.dma_start(wq, wst_d[:, 1])
            nc.sync.dma_start(wv, wst_d[:, 2])

            for j in range(NCHUNK):
                xr_parts, xi_parts = xparts

                # ---------- projections (PSUM-accumulated complex) ----------
                def proj(w2):
                    ps = mixps.tile([P, CH], f32, tag="mix")
                    for cc in range(CC):
                        nc.tensor.matmul(ps, w2[:, 0, cc],
                                         xr_parts[cc // 2][:, cc % 2],
                                         start=(cc == 0), stop=False)
                    for cc in range(CC):
                        nc.tensor.matmul(ps, w2[:, 1, cc],
                                         xi_parts[cc // 2][:, cc % 2],
                                         start=False, stop=(cc == CC - 1))
                    return ps

                c0, c1 = j * CH, (j + 1) * CH
                ps_k = proj(wk)
                nc.vector.tensor_copy(k_all[:, c0:c1], ps_k)

                ps_q = proj(wq)
                q_c = qc_p.tile([P, CH], f16, tag="qc")
                nc.scalar.copy(q_c, ps_q)

                # Q2 = [-qi; qr] = S @ Q+ via one signed-permutation matmul
                ps_q2 = mixps.tile([P, CH], f32, tag="mix")
                nc.tensor.matmul(ps_q2, s_perm, q_c, start=True, stop=True)
                q2_c = qc_p.tile([P, CH], f16, tag="q2c")
                nc.vector.tensor_copy(q2_c, ps_q2)

                ps_v = proj(wv)
                vt_c = qc_p.tile([P, CH], b16, tag="vtc")
                nc.vector.tensor_copy(vt_c, ps_v)
                # V+ [h2, t] -> natural [t, h2] blocks
                ps_vn = finps.tile([P, 4, H2], b16, tag="vno")
                for t4 in range(4):
                    nc.tensor.transpose(
                        ps_vn[:, t4], vt_c[:, t4 * P:(t4 + 1) * P], ident)
                nc.vector.tensor_copy(v_nat[:, j * 4:(j + 1) * 4], ps_vn)

                # prefetch next chunk's x behind this chunk's loads
                if j + 1 < NCHUNK:
                    xparts = load_chunk(j + 1)

                # deferred PV/sums of the previous chunk overlap proj above;
                # then the previous chunk can finalize
                while pending:
                    fn, args = pending.pop(0)
                    fn(*args)
                if prev_fin[0] is not None:
                    prev_fin[0]()
                    prev_fin[0] = None

                # ---------- scores / softmax / PV over tk blocks ----------
                ps_out = accps.tile([P, CH], f32, tag="outps")
                ps_sums = accps.tile([1, CH], f32, tag="sumps")
                nblk = 4 * (j + 1)
                # OCT groups; the LAST chunk's tail is split into QUADs to
                # shorten the exposed end-of-kernel latency chain
                if j == NCHUNK - 1:
                    groups = [list(range(g, g + 4)) for g in range(0, nblk - 4, 4)]
                    groups += [[nblk - 4, nblk - 3], [nblk - 2, nblk - 1]]
                else:
                    groups = [list(range(g, g + 4)) for g in range(0, nblk, 4)]

                def stage_a(blocks, q_c=q_c, q2_c=q2_c, j=j):
                    """scores + exits + squares + adds + ln/exp/exp for a
                    group of tk blocks. Returns the p tile."""
                    nb = len(blocks)
                    w = nb * CH
                    sq1 = elw_p.tile([P, w], f16, tag=f"sq1w{nb}")
                    sq2 = elw_p.tile([P, w], f16, tag=f"sq2w{nb}")
                    s_t = elw_p.tile([P, w], f16, tag=f"stw{nb}")
                    for b4, i in enumerate(blocks):
                        kT = k_all[:, i * P:(i + 1) * P]
                        ps_re = mixps.tile([P, CH], f32, tag="mix")
                        nc.tensor.matmul(ps_re, kT, q_c, start=True, stop=True)
                        ps_im = mixps.tile([P, CH], f32, tag="mix")
                        nc.tensor.matmul(ps_im, kT, q2_c, start=True, stop=True)
                        # each exit: ACT fused square (reads PSUM) or DVE
                        # scale-copy+mult, split to balance engine load
                        for ps_s, sq in ((ps_re, sq1), (ps_im, sq2)):
                            cs = slice(b4 * CH, (b4 + 1) * CH)
                            if (exit_ctr[0] % ACT_EXIT_DEN) < ACT_EXIT_NUM:
                                nc.scalar.activation(
                                    sq[:, cs], ps_s, AF.Square,
                                    bias=bias_zero, scale=0.25)
                            else:
                                e_s = im_p.tile([P, CH], f16, tag="es")
                                nc.vector.tensor_scalar_mul(e_s, ps_s, 0.25)
                                nc.vector.tensor_tensor(
                                    sq[:, cs], e_s, e_s, ALU.mult)
                            exit_ctr[0] += 1
                        if b4 % 2 == 1:  # QUAD-wide adds as halves complete
                            qs = slice((b4 - 1) * CH, (b4 + 1) * CH)
                            nc.gpsimd.tensor_tensor(
                                s_t[:, qs], sq1[:, qs], sq2[:, qs], ALU.add)

                    m_t = mt_p.tile([P, w], f32, tag=f"mtw{nb}")
                    # ln and first exp run in place over m_t (group-wide)
                    nc.scalar.activation(m_t, s_t, AF.Ln,
                                         bias=bias_eps, scale=1.0)
                    nc.scalar.activation(m_t, m_t, AF.Exp,
                                         bias=bias_lnc, scale=0.5)
                    p_t = p_p.tile([P, w], b16, tag=f"ptw{nb}")
                    nc.scalar.activation(p_t, m_t, AF.Exp,
                                         bias=bias_zero, scale=1.0)
                    return p_t

                def stage_b(blocks, p_t, first, last,
                            ps_out=ps_out, ps_sums=ps_sums, j=j):
                    """causal mask + row-sum + PV accumulation matmuls.
                    Masking here (LAG groups behind stage_a) keeps the
                    gpsimd FIFO from head-of-line blocking the adds."""
                    for b4, i in enumerate(blocks):
                        p_blk = p_t[:, b4 * CH:(b4 + 1) * CH]
                        if i >= 4 * j:  # diagonal: zero where tq < tk
                            nc.gpsimd.affine_select(
                                out=p_blk, in_=p_blk,
                                compare_op=ALU.is_ge,
                                fill=0.0,
                                base=j * CH - i * P,
                                pattern=[[1, CH]],
                                channel_multiplier=-1)
                    for b4, i in enumerate(blocks):
                        p_blk = p_t[:, b4 * CH:(b4 + 1) * CH]
                        st = first and b4 == 0
                        sp = last and b4 == len(blocks) - 1
                        nc.tensor.matmul(ps_sums, ones_col, p_blk,
                                         start=st, stop=sp)
                        nc.tensor.matmul(ps_out, v_nat[:, i], p_blk,
                                         start=st, stop=sp)

                for gi, blocks in enumerate(groups):
                    p_t = stage_a(blocks)
                    pending.append(
                        (stage_b,
                         (blocks, p_t, gi == 0, gi == len(groups) - 1)))
                    if len(pending) > LAG:
                        fn, args = pending.pop(0)
                        fn(*args)

                def finalize(j=j, ps_out=ps_out, ps_sums=ps_sums):
                    outT = fin_p.tile([P, CH], b16, tag="outT")
                    nc.vector.tensor_copy(outT, ps_out)
                    sums_sb = fin_p.tile([1, CH], f32, tag="sums")
                    nc.vector.tensor_copy(sums_sb, ps_sums)

                    ps_on = finps.tile([P, 4, H2], b16, tag="vno")
                    for t4 in range(4):
                        nc.tensor.transpose(
                            ps_on[:, t4], outT[:, t4 * P:(t4 + 1) * P], ident)
                    ps_rs = mixps.tile([P, CH], f32, tag="mix")
                    for t4 in range(4):
                        nc.tensor.matmul(ps_rs[:, t4:t4 + 1],
                                         sums_sb[0:1, t4 * P:(t4 + 1) * P],
                                         one1_f32, start=True, stop=True)
                    recip = fin_p.tile([P, 4], f32, tag="recip")
                    nc.vector.reciprocal(recip, ps_rs[:, 0:4])

                    onat = fin_p.tile([P, 4, H2], f32, tag="onat")
                    for t4 in range(4):
                        nc.vector.tensor_scalar_mul(
                            onat[:, t4], ps_on[:, t4], recip[:, t4:t4 + 1])
                    nc.scalar.dma_start(out_d[j], onat)

                prev_fin[0] = finalize

            # drain the last chunk
            while pending:
                fn, args = pending.pop(0)
                fn(*args)
            prev_fin[0]()

    _split_multiwaits(nc)
    return nc


def _split_multiwaits(nc):
    """This toolchain's walrus accepts at most ONE sync-wait per instruction;
    Tile's sem-assignment can attach several. Hoist all-but-one wait onto
    standalone InstEventSemaphore carriers (what bass's wait_ge emits)."""
    import concourse.mybir as mybir

    n_split = 0
    for f in nc.m.functions:
        for bb in f.blocks:
            out = []
            for inst in bb.instructions:
                si = inst.sync_info
                if si is not None and si.on_wait and len(si.on_wait) > 1:
                    waits = list(si.on_wait)
                    for w in waits[:-1]:
                        carrier = mybir.InstEventSemaphore(
                            name=f"{inst.name}_wsplit{n_split}", ins=[], outs=[])
                        carrier.engine = inst.engine
                        carrier.sync_info = mybir.SyncInfo(
                            on_wait=[w], on_update=[])
                        out.append(carrier)
                        n_split += 1
                    inst.sync_info = mybir.SyncInfo(
                        on_wait=[waits[-1]], on_update=list(si.on_update))
                out.append(inst)
            bb.instructions = out
    return n_split


def _host_prep(Wk_r, Wk_i, Wq_r, Wq_i, Wv_r, Wv_i):
    import ml_dtypes

    f16 = np.float16
    b16 = ml_dtypes.bfloat16

    def s1(wr, wi):
        return np.concatenate([wr, wi], axis=1)

    def s2(wr, wi):
        return np.concatenate([-wi, wr], axis=1)

    # [pair(3), 2, C, H2] -> partition-major [P, 3, 2, CC, H2]
    wst = np.stack([
        [s1(Wk_r, Wk_i), s2(Wk_r, Wk_i)],
        [s1(Wq_r, Wq_i), s2(Wq_r, Wq_i)],
        [s1(Wv_r, Wv_i), s2(Wv_r, Wv_i)],
    ]).astype(f16)
    wst = np.ascontiguousarray(
        wst.reshape(3, 2, C // P, P, H2).transpose(3, 0, 1, 2, 4))
    # S with S @ [qr; qi] = [-qi; qr]; matmul computes lhsT.T @ rhs so pass S^T
    s_mat = np.zeros((P, P), np.float32)
    for i in range(H):
        s_mat[i, H + i] = -1.0
        s_mat[H + i, i] = 1.0
    cper = np.ascontiguousarray(s_mat.T).astype(f16)
    cidn = np.ascontiguousarray(np.concatenate(
        [np.eye(P, dtype=np.float32), np.ones((P, 1), np.float32)],
        axis=1)).astype(b16)
    return wst, cper, cidn


def kernel(x_real, x_imag, Wk_r, Wk_i, Wq_r, Wq_i, Wv_r, Wv_i, _trace=False):
    global _BUILT
    from concourse.bass_utils import run_bass_kernel_spmd

    if _BUILT is None:
        _BUILT = _build()
    nc = _BUILT

    wst, cper, cidn = _host_prep(
        np.asarray(Wk_r), np.asarray(Wk_i), np.asarray(Wq_r),
        np.asarray(Wq_i), np.asarray(Wv_r), np.asarray(Wv_i))
    x_real = np.asarray(x_real, dtype=np.float32)
    x_imag = np.asarray(x_imag, dtype=np.float32)

    def xprep(xb):
        # (T, C) -> xT (C, T) -> [chunk, part, p, 2, t] partition-major, fp16
        return np.ascontiguousarray(
            xb.T.reshape(4, 2, P, NCHUNK, CH).transpose(3, 0, 2, 1, 4)
            .astype(np.float16))

    in_maps = [
        {
            "xT_real": xprep(x_real[b]),
            "xT_imag": xprep(x_imag[b]),
            "wstacks": wst,
            "consts_perm": cper,
            "consts_ident": cidn,
        }
        for b in range(B)
    ]
    res = run_bass_kernel_spmd(nc, in_maps, core_ids=list(range(B)),
                               trace=_trace)
    def unpack(pk):
        # [chunk, p, tb, h2] -> (T, H2)
        full = pk.transpose(0, 2, 1, 3).reshape(T, H2)
        return full[:, 0:H], full[:, H:H2]

    outs = [unpack(res.results[b]["out_pk"]) for b in range(B)]
    out_r = np.ascontiguousarray(np.stack([o[0] for o in outs]))
    out_i = np.ascontiguousarray(np.stack([o[1] for o in outs]))
    if _trace:
        kernel._last_results = res
    return out_r, out_i


# revision 17
# speedup vs baseline: 1.0909x; 1.0416x over previous
"""Complex-valued causal attention head on 8 Trainium2 NeuronCores.

Math (per batch element, fp32 reference):
    q/k/v = complex_linear(x, W*)          # contract C=1024 -> H=64
    wr + i*wi = q @ conj(k)^T              # contract H
    mag = sqrt(wr^2 + wi^2 + 1e-4) / sqrt(H)
    wei = softmax(causal_mask(mag))
    out = wei @ v   (real and imag parts separately)

Sharding: data-parallel over batch B=8 -> one batch element per core, weights
replicated, no collectives. Host-side prep is layout-only + 16-bit cast.

Per-core dataflow (T=2048, C=1024, H=64):
  - All matmul operands are 16-bit (PSUM accumulates fp32): fp32r moving
    operands stream at ~0.83 ns/row on the PE, 16-bit at ~0.42 ns/row, so
    this halves the tensor-engine critical path. Dtype split by range/
    precision: fp16 for x/W/k/q/scores^2 (small-range values where bf16's
    8-bit mantissa costs accuracy through the exp), bf16 for p/v/out^T
    (p reaches ~5e6 and sum(p*v) ~2e7 -> fp16 would overflow).
  - Squares are pre-scaled by 1/4 inside the exits (s' = |w|^2/16) for fp16
    headroom; the ln/exp chain absorbs the 1/16 in its bias.
  - Complex projections: pre-stacked weight pairs [Wr|Wi] / [-Wi|Wr] let PSUM
    accumulation do all the complex combines; outputs come out H-stacked and
    transposed: K+=[kr;ki], Q+=[qr;qi], V+=[vr;vi], each [128, T-chunk].
    Q2=[-qi;qr] is derived from Q+ with one signed-permutation matmul.
  - x and weights are DMA'd in fine-grained tiles (x in 4 cc-pair tiles per
    chunk, weights split per projection) so the first projection matmuls
    start as soon as the first slices land instead of after the full load.
  - Scores computed TRANSPOSED [tk, tq]: psRe = K+[:,tk]^T @ Q+,
    psIm = K+[:,tk]^T @ Q2 (sign dies in squaring); probabilities come out
    as the p^T operand that the PV matmul and ones-matmul row-sum need.
  - mag^2 exits from PSUM: re^2 via ACT Square(scale=1/4) or DVE
    tensor_scalar_mul+mult (split to balance engines; PSUM's single read
    port per engine forbids tensor_tensor(ps,ps)); im^2 via DVE. GPSIMD
    adds QUAD-wide; ACT runs ln/exp/exp OCT-wide (4 tk-blocks batched):
        p = exp(exp(0.5*ln(s' + eps/16) + ln(1/2)))
    (square/ln/exp share one ACT table set -> no table reloads).
  - Causal mask on diagonal blocks via in-place gpsimd affine_select (p:=0).
  - Row sums via ones-matmul on PE (M=1); PV accumulates out^T [h2, tq].
    The sums/PV matmuls for an OCT group are emitted LAG groups behind the
    score/softmax emissions, and the tail groups of each chunk are carried
    over past the next chunk's projection matmuls, so the PE never waits
    on the elementwise chain.
  - out^T is PE-transposed back to natural [t, h2]; the row 1/sum scaling
    rides the PSUM->SBUF copy via tensor_scalar_mul; DMA out.
"""

import numpy as np

B, T, C, H = 8, 2048, 1024, 64
H2 = 2 * H            # stacked real|imag head dim = 128
P = 128               # partitions
NCHUNK = 4            # T / 512
CH = T // NCHUNK      # 512 tq columns per chunk
TB = T // P           # 16 tk blocks
EPS = 1e-4
QUAD = 2 * CH         # gpsimd add width (2 tk-blocks)
OCT = 4 * CH          # ACT chain width (4 tk-blocks)
LAG = 2               # OCT groups of score->softmax in flight before PV
ACT_EXIT_NUM, ACT_EXIT_DEN = 3, 8   # fraction of ALL exits taken by ACT

_BUILT = None


def _build(loop_n=None):
    import contextlib

    import concourse.bass as bass
    import concourse.mybir as mybir
    import concourse.tile as tile

    f32 = mybir.dt.float32
    f16 = mybir.dt.float16
    b16 = mybir.dt.bfloat16
    AF = mybir.ActivationFunctionType
    ALU = mybir.AluOpType

    nc = bass.Bass(trn_type="TRN2")

    # x pre-transposed AND partition-major: [chunk, part(4), p, 2, t] so each
    # partition reads one contiguous run per sub-tile DMA
    xr_d = nc.dram_tensor("xT_real", [NCHUNK, 4, P, 2, CH], f16, kind="ExternalInput").ap()
    xi_d = nc.dram_tensor("xT_imag", [NCHUNK, 4, P, 2, CH], f16, kind="ExternalInput").ap()
    # weight stacks [P, pair(3: K,Q,V), 2, CC, H2]; pair p: [:,p,0]=S1, [:,p,1]=S2
    wst_d = nc.dram_tensor("wstacks", [P, 3, 2, C // P, H2], f16, kind="ExternalInput").ap()
    # consts16: [:, :128]=S_T perm fp16 ; consts_b: ident bf16 + ones bf16
    cper_d = nc.dram_tensor("consts_perm", [P, P], f16, kind="ExternalInput").ap()
    cidn_d = nc.dram_tensor("consts_ident", [P, P + 1], b16, kind="ExternalInput").ap()

    # packed output [chunk, p, tb, h2]; host unpacks to (T, H) r/i halves
    out_d = nc.dram_tensor("out_pk", [NCHUNK, P, 4, H2], f32, kind="ExternalOutput").ap()

    CC = C // P  # 8 contraction chunks

    with tile.TileContext(nc) as tc:
        ctx = contextlib.ExitStack()
        with ctx:
            if loop_n is not None:
                ctx.enter_context(tc.For_i(0, loop_n, 1))
            singles = ctx.enter_context(tc.tile_pool(name="singles", bufs=1))
            xt_p = ctx.enter_context(tc.tile_pool(name="xt", bufs=2))
            qc_p = ctx.enter_context(tc.tile_pool(name="qc", bufs=2))
            elw_p = ctx.enter_context(tc.tile_pool(name="elw", bufs=3))
            mt_p = ctx.enter_context(tc.tile_pool(name="mtp", bufs=2))
            im_p = ctx.enter_context(tc.tile_pool(name="imp", bufs=6))
            p_p = ctx.enter_context(tc.tile_pool(name="pp", bufs=LAG + 2))
            fin_p = ctx.enter_context(tc.tile_pool(name="fin", bufs=2))

            # PSUM budget is 8 banks (2KB/partition each), reserved statically:
            # mixps 5 + accps(out+sums) 2 + finps 1 = 8. One shared 5-deep
            # [P, CH] f32 ring serves projection accumulators, score tiles
            # and the tiny row-sum-transpose output: proj psum is idle during
            # the score phase and vice versa, and the deep ring lets the
            # PE run several score matmuls ahead of the exits.
            mixps = ctx.enter_context(tc.tile_pool(name="mixps", bufs=5, space="PSUM"))
            accps = ctx.enter_context(tc.tile_pool(name="accps", bufs=1, space="PSUM"))
            finps = ctx.enter_context(tc.tile_pool(name="finps", bufs=1, space="PSUM"))

            # ---- weights / constants (fine-grained so proj-K starts early) ----
            wk = singles.tile([P, 2, CC, H2], f16)
            wq = singles.tile([P, 2, CC, H2], f16)
            wv = singles.tile([P, 2, CC, H2], f16)
            nc.sync.dma_start(wk, wst_d[:, 0])

            s_perm = singles.tile([P, P], f16)
            nc.scalar.dma_start(s_perm, cper_d)
            cidn = singles.tile([P, P + 1], b16)
            nc.scalar.dma_start(cidn, cidn_d)
            ident = cidn[:, 0:P]
            ones_col = cidn[:, P:P + 1]

            one1_f32 = singles.tile([1, 1], f32)
            nc.vector.memset(one1_f32, 1.0)
            bias_eps = singles.tile([P, 1], f32)
            nc.vector.memset(bias_eps, EPS / 16.0)
            bias_lnc = singles.tile([P, 1], f32)
            nc.vector.memset(bias_lnc, float(np.log(0.5)))
            bias_zero = singles.tile([P, 1], f32)
            nc.vector.memset(bias_zero, 0.0)

            # ---- persistent per-batch buffers ----
            k_all = singles.tile([P, T], f16)        # K+ = [kr^T; ki^T]
            v_nat = singles.tile([P, TB, H2], b16)   # V natural [t, h2] blocks

            exit_ctr = [0]   # global counter for ACT/DVE re^2 exit split
            pend_a2 = []     # (b_fn, blocks, s_t, first, last) awaiting chain
            pend_b = []      # (b_fn, blocks, p_t, first, last) awaiting PV
            prev_fin = [None]

            def stage_a2(blocks, s_t):
                """ln/exp/exp chain -> p tile. Runs one group behind
                stage_a1 so the chain (which waits on the gpsimd adds)
                never head-of-line blocks the next group's Square exits
                in the ACT FIFO."""
                nb = len(blocks)
                w = nb * CH
                m_t = mt_p.tile([P, w], f32, tag=f"mtw{nb}")
                # ln and first exp run in place over m_t (group-wide)
                nc.scalar.activation(m_t, s_t, AF.Ln,
                                     bias=bias_eps, scale=1.0)
                nc.scalar.activation(m_t, m_t, AF.Exp,
                                     bias=bias_lnc, scale=0.5)
                p_t = p_p.tile([P, w], b16, tag=f"ptw{nb}")
                nc.scalar.activation(p_t, m_t, AF.Exp,
                                     bias=bias_zero, scale=1.0)
                return p_t

            def flush(keep_a2, keep_b):
                while len(pend_a2) > keep_a2:
                    b_fn, blocks, s_t, first, last = pend_a2.pop(0)
                    p_t = stage_a2(blocks, s_t)
                    pend_b.append((b_fn, blocks, p_t, first, last))
                while len(pend_b) > keep_b:
                    b_fn, blocks, p_t, first, last = pend_b.pop(0)
                    b_fn(blocks, p_t, first, last)

            def load_chunk(j):
                # chunk 0 is latency-critical: split xr/xi across the two
                # HWDGE queues. Later chunks are prefetch -> keep them off
                # the scalar queue so its engine (ACT) isn't burdened with
                # descriptor-generation instructions.
                xi_eng = nc.scalar if j == 0 else nc.sync
                xr_parts, xi_parts = [], []
                for part in range(4):
                    xr_t = xt_p.tile([P, 2, CH], f16, tag=f"xtr{part}")
                    nc.sync.dma_start(xr_t, xr_d[j, part])
                    xr_parts.append(xr_t)
                for part in range(4):
                    xi_t = xt_p.tile([P, 2, CH], f16, tag=f"xti{part}")
                    xi_eng.dma_start(xi_t, xi_d[j, part])
                    xi_parts.append(xi_t)
                return xr_parts, xi_parts

            # chunk-0 loads + remaining weights (emitted after, so wk/x win
            # the queues and proj-K starts as early as possible)
            xparts = load_chunk(0)
            nc.sync.dma_start(wq, wst_d[:, 1])
            nc.sync.dma_start(wv, wst_d[:, 2])

            for j in range(NCHUNK):
                xr_parts, xi_parts = xparts

                # ---------- projections (PSUM-accumulated complex) ----------
                def proj(w2):
                    ps = mixps.tile([P, CH], f32, tag="mix")
                    for cc in range(CC):
                        nc.tensor.matmul(ps, w2[:, 0, cc],
                                         xr_parts[cc // 2][:, cc % 2],
                                         start=(cc == 0), stop=False)
                    for cc in range(CC):
                        nc.tensor.matmul(ps, w2[:, 1, cc],
                                         xi_parts[cc // 2][:, cc % 2],
                                         start=False, stop=(cc == CC - 1))
                    return ps

                c0, c1 = j * CH, (j + 1) * CH
                ps_k = proj(wk)
                nc.vector.tensor_copy(k_all[:, c0:c1], ps_k)

                ps_q = proj(wq)
                q_c = qc_p.tile([P, CH], f16, tag="qc")
                nc.scalar.copy(q_c, ps_q)

                # Q2 = [-qi; qr] = S @ Q+ via one signed-permutation matmul
                ps_q2 = mixps.tile([P, CH], f32, tag="mix")
                nc.tensor.matmul(ps_q2, s_perm, q_c, start=True, stop=True)
                q2_c = qc_p.tile([P, CH], f16, tag="q2c")
                nc.vector.tensor_copy(q2_c, ps_q2)

                ps_v = proj(wv)
                vt_c = qc_p.tile([P, CH], b16, tag="vtc")
                nc.vector.tensor_copy(vt_c, ps_v)
                # V+ [h2, t] -> natural [t, h2] blocks
                ps_vn = finps.tile([P, 4, H2], b16, tag="vno")
                for t4 in range(4):
                    nc.tensor.transpose(
                        ps_vn[:, t4], vt_c[:, t4 * P:(t4 + 1) * P], ident)
                nc.vector.tensor_copy(v_nat[:, j * 4:(j + 1) * 4], ps_vn)

                # prefetch next chunk's x behind this chunk's loads
                if j + 1 < NCHUNK:
                    xparts = load_chunk(j + 1)

                # deferred chain/PV of the previous chunk overlap proj above;
                # then the previous chunk can finalize
                flush(0, 0)
                if prev_fin[0] is not None:
                    prev_fin[0]()
                    prev_fin[0] = None

                # ---------- scores / softmax / PV over tk blocks ----------
                ps_out = accps.tile([P, CH], f32, tag="outps")
                ps_sums = accps.tile([1, CH], f32, tag="sumps")
                nblk = 4 * (j + 1)
                # OCT groups; the LAST chunk's tail is split into QUADs to
                # shorten the exposed end-of-kernel latency chain
                if j == NCHUNK - 1:
                    groups = [list(range(g, g + 4)) for g in range(0, nblk - 4, 4)]
                    groups += [[nblk - 4, nblk - 3], [nblk - 2, nblk - 1]]
                else:
                    groups = [list(range(g, g + 4)) for g in range(0, nblk, 4)]

                def stage_a1(blocks, q_c=q_c, q2_c=q2_c):
                    """scores + exits + squares + adds for a group of tk
                    blocks. Returns the summed |w|^2/16 tile."""
                    nb = len(blocks)
                    w = nb * CH
                    sq1 = elw_p.tile([P, w], f16, tag=f"sq1w{nb}")
                    sq2 = elw_p.tile([P, w], f16, tag=f"sq2w{nb}")
                    s_t = elw_p.tile([P, w], f16, tag=f"stw{nb}")
                    for b4, i in enumerate(blocks):
                        kT = k_all[:, i * P:(i + 1) * P]
                        ps_re = mixps.tile([P, CH], f32, tag="mix")
                        nc.tensor.matmul(ps_re, kT, q_c, start=True, stop=True)
                        ps_im = mixps.tile([P, CH], f32, tag="mix")
                        nc.tensor.matmul(ps_im, kT, q2_c, start=True, stop=True)
                        # each exit: ACT fused square (reads PSUM) or DVE
                        # scale-copy+mult, split to balance engine load
                        for ps_s, sq in ((ps_re, sq1), (ps_im, sq2)):
                            cs = slice(b4 * CH, (b4 + 1) * CH)
                            if (exit_ctr[0] % ACT_EXIT_DEN) < ACT_EXIT_NUM:
                                nc.scalar.activation(
                                    sq[:, cs], ps_s, AF.Square,
                                    bias=bias_zero, scale=0.25)
                            else:
                                e_s = im_p.tile([P, CH], f16, tag="es")
                                nc.vector.tensor_scalar_mul(e_s, ps_s, 0.25)
                                nc.vector.tensor_tensor(
                                    sq[:, cs], e_s, e_s, ALU.mult)
                            exit_ctr[0] += 1
                        if b4 % 2 == 1:  # QUAD-wide adds as halves complete
                            qs = slice((b4 - 1) * CH, (b4 + 1) * CH)
                            nc.gpsimd.tensor_tensor(
                                s_t[:, qs], sq1[:, qs], sq2[:, qs], ALU.add)
                    return s_t

                def stage_b(blocks, p_t, first, last,
                            ps_out=ps_out, ps_sums=ps_sums, j=j):
                    """causal mask + row-sum + PV accumulation matmuls.
                    Masking here (LAG groups behind stage_a) keeps the
                    gpsimd FIFO from head-of-line blocking the adds."""
                    for b4, i in enumerate(blocks):
                        p_blk = p_t[:, b4 * CH:(b4 + 1) * CH]
                        if i >= 4 * j:  # diagonal: zero where tq < tk
                            nc.gpsimd.affine_select(
                                out=p_blk, in_=p_blk,
                                compare_op=ALU.is_ge,
                                fill=0.0,
                                base=j * CH - i * P,
                                pattern=[[1, CH]],
                                channel_multiplier=-1)
                    for b4, i in enumerate(blocks):
                        p_blk = p_t[:, b4 * CH:(b4 + 1) * CH]
                        st = first and b4 == 0
                        sp = last and b4 == len(blocks) - 1
                        nc.tensor.matmul(ps_sums, ones_col, p_blk,
                                         start=st, stop=sp)
                        nc.tensor.matmul(ps_out, v_nat[:, i], p_blk,
                                         start=st, stop=sp)

                for gi, blocks in enumerate(groups):
                    s_t = stage_a1(blocks)
                    pend_a2.append(
                        (stage_b, blocks, s_t, gi == 0,
                         gi == len(groups) - 1))
                    flush(1, 1)

                def finalize(j=j, ps_out=ps_out, ps_sums=ps_sums):
                    outT = fin_p.tile([P, CH], b16, tag="outT")
                    nc.vector.tensor_copy(outT, ps_out)
                    sums_sb = fin_p.tile([1, CH], f32, tag="sums")
                    nc.vector.tensor_copy(sums_sb, ps_sums)

                    ps_on = finps.tile([P, 4, H2], b16, tag="vno")
                    for t4 in range(4):
                        nc.tensor.transpose(
                            ps_on[:, t4], outT[:, t4 * P:(t4 + 1) * P], ident)
                    ps_rs = mixps.tile([P, CH], f32, tag="mix")
                    for t4 in range(4):
                        nc.tensor.matmul(ps_rs[:, t4:t4 + 1],
                                         sums_sb[0:1, t4 * P:(t4 + 1) * P],
                                         one1_f32, start=True, stop=True)
                    recip = fin_p.tile([P, 4], f32, tag="recip")
                    nc.vector.reciprocal(recip, ps_rs[:, 0:4])

                    onat = fin_p.tile([P, 4, H2], f32, tag="onat")
                    for t4 in range(4):
                        nc.vector.tensor_scalar_mul(
                            onat[:, t4], ps_on[:, t4], recip[:, t4:t4 + 1])
                    nc.scalar.dma_start(out_d[j], onat)

                prev_fin[0] = finalize

            # drain the last chunk
            flush(0, 0)
            prev_fin[0]()

    _split_multiwaits(nc)
    return nc


def _split_multiwaits(nc):
    """This toolchain's walrus accepts at most ONE sync-wait per instruction;
    Tile's sem-assignment can attach several. Hoist all-but-one wait onto
    standalone InstEventSemaphore carriers (what bass's wait_ge emits)."""
    import concourse.mybir as mybir

    n_split = 0
    for f in nc.m.functions:
        for bb in f.blocks:
            out = []
            for inst in bb.instructions:
                si = inst.sync_info
                if si is not None and si.on_wait and len(si.on_wait) > 1:
                    waits = list(si.on_wait)
                    for w in waits[:-1]:
                        carrier = mybir.InstEventSemaphore(
                            name=f"{inst.name}_wsplit{n_split}", ins=[], outs=[])
                        carrier.engine = inst.engine
                        carrier.sync_info = mybir.SyncInfo(
                            on_wait=[w], on_update=[])
                        out.append(carrier)
                        n_split += 1
                    inst.sync_info = mybir.SyncInfo(
                        on_wait=[waits[-1]], on_update=list(si.on_update))
                out.append(inst)
            bb.instructions = out
    return n_split


def _host_prep(Wk_r, Wk_i, Wq_r, Wq_i, Wv_r, Wv_i):
    import ml_dtypes

    f16 = np.float16
    b16 = ml_dtypes.bfloat16

    def s1(wr, wi):
        return np.concatenate([wr, wi], axis=1)

    def s2(wr, wi):
        return np.concatenate([-wi, wr], axis=1)

    # [pair(3), 2, C, H2] -> partition-major [P, 3, 2, CC, H2]
    wst = np.stack([
        [s1(Wk_r, Wk_i), s2(Wk_r, Wk_i)],
        [s1(Wq_r, Wq_i), s2(Wq_r, Wq_i)],
        [s1(Wv_r, Wv_i), s2(Wv_r, Wv_i)],
    ]).astype(f16)
    wst = np.ascontiguousarray(
        wst.reshape(3, 2, C // P, P, H2).transpose(3, 0, 1, 2, 4))
    # S with S @ [qr; qi] = [-qi; qr]; matmul computes lhsT.T @ rhs so pass S^T
    s_mat = np.zeros((P, P), np.float32)
    for i in range(H):
        s_mat[i, H + i] = -1.0
        s_mat[H + i, i] = 1.0
    cper = np.ascontiguousarray(s_mat.T).astype(f16)
    cidn = np.ascontiguousarray(np.concatenate(
        [np.eye(P, dtype=np.float32), np.ones((P, 1), np.float32)],
        axis=1)).astype(b16)
    return wst, cper, cidn


def kernel(x_real, x_imag, Wk_r, Wk_i, Wq_r, Wq_i, Wv_r, Wv_i, _trace=False):
    global _BUILT
    from concourse.bass_utils import run_bass_kernel_spmd

    if _BUILT is None:
        _BUILT = _build()
    nc = _BUILT

    wst, cper, cidn = _host_prep(
        np.asarray(Wk_r), np.asarray(Wk_i), np.asarray(Wq_r),
        np.asarray(Wq_i), np.asarray(Wv_r), np.asarray(Wv_i))
    x_real = np.asarray(x_real, dtype=np.float32)
    x_imag = np.asarray(x_imag, dtype=np.float32)

    def xprep(xb):
        # (T, C) -> xT (C, T) -> [chunk, part, p, 2, t] partition-major, fp16
        return np.ascontiguousarray(
            xb.T.reshape(4, 2, P, NCHUNK, CH).transpose(3, 0, 2, 1, 4)
            .astype(np.float16))

    in_maps = [
        {
            "xT_real": xprep(x_real[b]),
            "xT_imag": xprep(x_imag[b]),
            "wstacks": wst,
            "consts_perm": cper,
            "consts_ident": cidn,
        }
        for b in range(B)
    ]
    res = run_bass_kernel_spmd(nc, in_maps, core_ids=list(range(B)),
                               trace=_trace)
    def unpack(pk):
        # [chunk, p, tb, h2] -> (T, H2)
        full = pk.transpose(0, 2, 1, 3).reshape(T, H2)
        return full[:, 0:H], full[:, H:H2]

    outs = [unpack(res.results[b]["out_pk"]) for b in range(B)]
    out_r = np.ascontiguousarray(np.stack([o[0] for o in outs]))
    out_i = np.ascontiguousarray(np.stack([o[1] for o in outs]))
    if _trace:
        kernel._last_results = res
    return out_r, out_i


# revision 18
# speedup vs baseline: 1.1532x; 1.0571x over previous
"""Complex-valued causal attention head on 8 Trainium2 NeuronCores.

Math (per batch element, fp32 reference):
    q/k/v = complex_linear(x, W*)          # contract C=1024 -> H=64
    wr + i*wi = q @ conj(k)^T              # contract H
    mag = sqrt(wr^2 + wi^2 + 1e-4) / sqrt(H)
    wei = softmax(causal_mask(mag))
    out = wei @ v   (real and imag parts separately)

Sharding: data-parallel over batch B=8 -> one batch element per core, weights
replicated, no collectives. Host-side prep is layout-only + 16-bit cast.

Per-core dataflow (T=2048, C=1024, H=64):
  - All matmul operands are 16-bit (PSUM accumulates fp32): fp32r moving
    operands stream at ~0.83 ns/row on the PE, 16-bit at ~0.42 ns/row, so
    this halves the tensor-engine critical path. Dtype split by range/
    precision: fp16 for x/W/k/q/scores^2 (small-range values where bf16's
    8-bit mantissa costs accuracy through the exp), bf16 for p/v/out^T
    (p reaches ~5e6 and sum(p*v) ~2e7 -> fp16 would overflow).
  - Squares are pre-scaled by 1/4 inside the exits (s' = |w|^2/16) for fp16
    headroom; the ln/exp chain absorbs the 1/16 in its bias.
  - Complex projections: pre-stacked weight pairs [Wr|Wi] / [-Wi|Wr] let PSUM
    accumulation do all the complex combines; outputs come out H-stacked and
    transposed: K+=[kr;ki], Q+=[qr;qi], V+=[vr;vi], each [128, T-chunk].
    Q2=[-qi;qr] is derived from Q+ with one signed-permutation matmul.
  - x and weights are DMA'd in fine-grained tiles (x in 4 cc-pair tiles per
    chunk, weights split per projection) so the first projection matmuls
    start as soon as the first slices land instead of after the full load.
  - Scores computed TRANSPOSED [tk, tq]: psRe = K+[:,tk]^T @ Q+,
    psIm = K+[:,tk]^T @ Q2 (sign dies in squaring); probabilities come out
    as the p^T operand that the PV matmul and ones-matmul row-sum need.
  - mag^2 exits from PSUM: re^2 via ACT Square(scale=1/4) or DVE
    tensor_scalar_mul+mult (split to balance engines; PSUM's single read
    port per engine forbids tensor_tensor(ps,ps)); im^2 via DVE. GPSIMD
    adds QUAD-wide; ACT runs ln/exp/exp OCT-wide (4 tk-blocks batched):
        p = exp(exp(0.5*ln(s' + eps/16) + ln(1/2)))
    (square/ln/exp share one ACT table set -> no table reloads).
  - Causal mask on diagonal blocks via in-place gpsimd affine_select (p:=0).
  - Row sums via ones-matmul on PE (M=1); PV accumulates out^T [h2, tq].
    The sums/PV matmuls for an OCT group are emitted LAG groups behind the
    score/softmax emissions, and the tail groups of each chunk are carried
    over past the next chunk's projection matmuls, so the PE never waits
    on the elementwise chain.
  - out^T is PE-transposed back to natural [t, h2]; the row 1/sum scaling
    rides the PSUM->SBUF copy via tensor_scalar_mul; DMA out.
"""

import numpy as np

B, T, C, H = 8, 2048, 1024, 64
H2 = 2 * H            # stacked real|imag head dim = 128
P = 128               # partitions
NCHUNK = 4            # T / 512
CH = T // NCHUNK      # 512 tq columns per chunk
TB = T // P           # 16 tk blocks
EPS = 1e-4
QUAD = 2 * CH         # gpsimd add width (2 tk-blocks)
OCT = 4 * CH          # ACT chain width (4 tk-blocks)
LAG = 2               # OCT groups of score->softmax in flight before PV
ACT_EXIT_NUM, ACT_EXIT_DEN = 3, 8   # fraction of ALL exits taken by ACT

_BUILT = None


def _build(loop_n=None):
    import contextlib

    import concourse.bass as bass
    import concourse.mybir as mybir
    import concourse.tile as tile

    f32 = mybir.dt.float32
    f16 = mybir.dt.float16
    b16 = mybir.dt.bfloat16
    AF = mybir.ActivationFunctionType
    ALU = mybir.AluOpType

    nc = bass.Bass(trn_type="TRN2")

    # x pre-transposed AND partition-major: [chunk, part(4), p, 2, t] so each
    # partition reads one contiguous run per sub-tile DMA
    xr_d = nc.dram_tensor("xT_real", [NCHUNK, 4, P, 2, CH], f16, kind="ExternalInput").ap()
    xi_d = nc.dram_tensor("xT_imag", [NCHUNK, 4, P, 2, CH], f16, kind="ExternalInput").ap()
    # weight stacks [P, pair(3: K,Q,V), 2, CC, H2]; pair p: [:,p,0]=S1, [:,p,1]=S2
    wst_d = nc.dram_tensor("wstacks", [P, 3, 2, C // P, H2], f16, kind="ExternalInput").ap()
    # consts16: [:, :128]=S_T perm fp16 ; consts_b: ident bf16 + ones bf16
    cper_d = nc.dram_tensor("consts_perm", [P, P], f16, kind="ExternalInput").ap()
    cidn_d = nc.dram_tensor("consts_ident", [P, P + 1], b16, kind="ExternalInput").ap()

    # packed output [chunk, p, tb, h2]; host unpacks to (T, H) r/i halves
    out_d = nc.dram_tensor("out_pk", [NCHUNK, P, 4, H2], f32, kind="ExternalOutput").ap()

    CC = C // P  # 8 contraction chunks

    with tile.TileContext(nc) as tc:
        ctx = contextlib.ExitStack()
        with ctx:
            if loop_n is not None:
                ctx.enter_context(tc.For_i(0, loop_n, 1))
            singles = ctx.enter_context(tc.tile_pool(name="singles", bufs=1))
            xt_p = ctx.enter_context(tc.tile_pool(name="xt", bufs=2))
            qc_p = ctx.enter_context(tc.tile_pool(name="qc", bufs=2))
            elw_p = ctx.enter_context(tc.tile_pool(name="elw", bufs=3))
            mt_p = ctx.enter_context(tc.tile_pool(name="mtp", bufs=2))
            im_p = ctx.enter_context(tc.tile_pool(name="imp", bufs=6))
            p_p = ctx.enter_context(tc.tile_pool(name="pp", bufs=LAG + 2))
            fin_p = ctx.enter_context(tc.tile_pool(name="fin", bufs=2))

            # PSUM budget is 8 banks (2KB/partition each), reserved statically:
            # projps 1 + scmix 4 + accps(out+sums) 2 + finps 1 = 8.
            # The 4-deep scmix ring holds score tiles (and the tiny row-sum
            # transpose at chunk end); projections accumulate in their own
            # bank because they are now interleaved INTO the score phase.
            projps = ctx.enter_context(tc.tile_pool(name="projps", bufs=1, space="PSUM"))
            scmix = ctx.enter_context(tc.tile_pool(name="scmix", bufs=4, space="PSUM"))
            accps = ctx.enter_context(tc.tile_pool(name="accps", bufs=1, space="PSUM"))
            finps = ctx.enter_context(tc.tile_pool(name="finps", bufs=1, space="PSUM"))

            # ---- weights / constants (fine-grained so proj-K starts early) ----
            wk = singles.tile([P, 2, CC, H2], f16)
            wq = singles.tile([P, 2, CC, H2], f16)
            wv = singles.tile([P, 2, CC, H2], f16)
            nc.sync.dma_start(wk, wst_d[:, 0])

            s_perm = singles.tile([P, P], f16)
            nc.scalar.dma_start(s_perm, cper_d)
            cidn = singles.tile([P, P + 1], b16)
            nc.scalar.dma_start(cidn, cidn_d)
            ident = cidn[:, 0:P]
            ones_col = cidn[:, P:P + 1]

            one1_f32 = singles.tile([1, 1], f32)
            nc.vector.memset(one1_f32, 1.0)
            bias_eps = singles.tile([P, 1], f32)
            nc.vector.memset(bias_eps, EPS / 16.0)
            bias_lnc = singles.tile([P, 1], f32)
            nc.vector.memset(bias_lnc, float(np.log(0.5)))
            bias_zero = singles.tile([P, 1], f32)
            nc.vector.memset(bias_zero, 0.0)

            # ---- persistent per-batch buffers ----
            k_all = singles.tile([P, T], f16)        # K+ = [kr^T; ki^T]
            v_nat = singles.tile([P, TB, H2], b16)   # V natural [t, h2] blocks

            exit_ctr = [0]   # global counter for ACT/DVE re^2 exit split
            pend_a2 = []     # (b_fn, blocks, s_t, first, last) awaiting chain
            pend_b = []      # (b_fn, blocks, p_t, first, last, fin) awaiting PV

            def stage_a2(blocks, s_t):
                """ln/exp/exp chain -> p tile. Runs one group behind
                stage_a1 so the chain (which waits on the gpsimd adds)
                never head-of-line blocks the next group's Square exits
                in the ACT FIFO."""
                nb = len(blocks)
                w = nb * CH
                m_t = mt_p.tile([P, w], f32, tag=f"mtw{nb}")
                # ln and first exp run in place over m_t (group-wide)
                nc.scalar.activation(m_t, s_t, AF.Ln,
                                     bias=bias_eps, scale=1.0)
                nc.scalar.activation(m_t, m_t, AF.Exp,
                                     bias=bias_lnc, scale=0.5)
                p_t = p_p.tile([P, w], b16, tag=f"ptw{nb}")
                nc.scalar.activation(p_t, m_t, AF.Exp,
                                     bias=bias_zero, scale=1.0)
                return p_t

            def flush(keep_a2, keep_b):
                while len(pend_a2) > keep_a2:
                    b_fn, blocks, s_t, first, last, fin = pend_a2.pop(0)
                    p_t = stage_a2(blocks, s_t)
                    pend_b.append((b_fn, blocks, p_t, first, last, fin))
                while len(pend_b) > keep_b:
                    b_fn, blocks, p_t, first, last, fin = pend_b.pop(0)
                    b_fn(blocks, p_t, first, last)
                    if fin is not None:
                        fin()

            def load_chunk(j):
                # chunk 0 is latency-critical: split xr/xi across the two
                # HWDGE queues. Later chunks are prefetch -> keep them off
                # the scalar queue so its engine (ACT) isn't burdened with
                # descriptor-generation instructions.
                xi_eng = nc.scalar if j == 0 else nc.sync
                xr_parts, xi_parts = [], []
                for part in range(4):
                    xr_t = xt_p.tile([P, 2, CH], f16, tag=f"xtr{part}")
                    nc.sync.dma_start(xr_t, xr_d[j, part])
                    xr_parts.append(xr_t)
                for part in range(4):
                    xi_t = xt_p.tile([P, 2, CH], f16, tag=f"xti{part}")
                    xi_eng.dma_start(xi_t, xi_d[j, part])
                    xi_parts.append(xi_t)
                return xr_parts, xi_parts

            # projection pipeline for one chunk, as a generator: the main
            # loop steps it between score groups so projection matmuls fill
            # PE gaps while the elementwise engines digest scores (keeps the
            # PE HAM-warm and never idle-waiting on PSUM exits)
            def gen_proj(j, xr_parts, xi_parts, prod):
                c0 = j * CH

                def halves(w2, ps):
                    for half, parts in ((0, xr_parts), (1, xi_parts)):
                        for cc in range(CC):
                            nc.tensor.matmul(
                                ps, w2[:, half, cc],
                                parts[cc // 2][:, cc % 2],
                                start=(half == 0 and cc == 0),
                                stop=(half == 1 and cc == CC - 1))
                            if cc == 3:
                                yield
                        yield

                ps = projps.tile([P, CH], f32, tag="projps")
                yield from halves(wk, ps)
                nc.vector.tensor_copy(k_all[:, c0:c0 + CH], ps)
                yield
                ps = projps.tile([P, CH], f32, tag="projps")
                yield from halves(wq, ps)
                q_c = qc_p.tile([P, CH], f16, tag="qc")
                nc.scalar.copy(q_c, ps)
                # Q2 = [-qi; qr] = S @ Q+ via one signed-permutation matmul
                ps_q2 = scmix.tile([P, CH], f32, tag="mix")
                nc.tensor.matmul(ps_q2, s_perm, q_c, start=True, stop=True)
                q2_c = qc_p.tile([P, CH], f16, tag="q2c")
                nc.vector.tensor_copy(q2_c, ps_q2)
                prod["q"] = (q_c, q2_c)
                yield
                ps = projps.tile([P, CH], f32, tag="projps")
                yield from halves(wv, ps)
                vt_c = qc_p.tile([P, CH], b16, tag="vtc")
                nc.vector.tensor_copy(vt_c, ps)
                yield
                # V+ [h2, t] -> natural [t, h2] blocks
                ps_vn = finps.tile([P, 4, H2], b16, tag="vno")
                for t4 in range(4):
                    nc.tensor.transpose(
                        ps_vn[:, t4], vt_c[:, t4 * P:(t4 + 1) * P], ident)
                nc.vector.tensor_copy(v_nat[:, j * 4:(j + 1) * 4], ps_vn)

            # chunk-0 loads + remaining weights (emitted after, so wk/x win
            # the queues and proj-K starts as early as possible); chunk 0's
            # projections run un-interleaved
            xparts = load_chunk(0)
            nc.sync.dma_start(wq, wst_d[:, 1])
            nc.sync.dma_start(wv, wst_d[:, 2])
            prods = {}
            for _ in gen_proj(0, xparts[0], xparts[1], prods):
                pass

            for j in range(NCHUNK):
                q_c, q2_c = prods.pop("q")

                # prefetch next chunk's x; its projections interleave below
                if j + 1 < NCHUNK:
                    xparts = load_chunk(j + 1)
                    prods = {}
                    gen = gen_proj(j + 1, xparts[0], xparts[1], prods)
                else:
                    gen = None

                # ---------- scores / softmax / PV over tk blocks ----------
                ps_out = accps.tile([P, CH], f32, tag="outps")
                ps_sums = accps.tile([1, CH], f32, tag="sumps")
                nblk = 4 * (j + 1)
                # OCT groups; the LAST chunk's tail is split into QUADs to
                # shorten the exposed end-of-kernel latency chain
                if j == NCHUNK - 1:
                    groups = [list(range(g, g + 4)) for g in range(0, nblk - 4, 4)]
                    groups += [[nblk - 4, nblk - 3], [nblk - 2, nblk - 1]]
                else:
                    groups = [list(range(g, g + 4)) for g in range(0, nblk, 4)]

                def stage_a1(blocks, q_c=q_c, q2_c=q2_c):
                    """scores + exits + squares + adds for a group of tk
                    blocks. Returns the summed |w|^2/16 tile."""
                    nb = len(blocks)
                    w = nb * CH
                    sq1 = elw_p.tile([P, w], f16, tag=f"sq1w{nb}")
                    sq2 = elw_p.tile([P, w], f16, tag=f"sq2w{nb}")
                    s_t = elw_p.tile([P, w], f16, tag=f"stw{nb}")
                    for b4, i in enumerate(blocks):
                        kT = k_all[:, i * P:(i + 1) * P]
                        ps_re = scmix.tile([P, CH], f32, tag="mix")
                        nc.tensor.matmul(ps_re, kT, q_c, start=True, stop=True)
                        ps_im = scmix.tile([P, CH], f32, tag="mix")
                        nc.tensor.matmul(ps_im, kT, q2_c, start=True, stop=True)
                        # each exit: ACT fused square (reads PSUM) or DVE
                        # scale-copy+mult, split to balance engine load
                        for ps_s, sq in ((ps_re, sq1), (ps_im, sq2)):
                            cs = slice(b4 * CH, (b4 + 1) * CH)
                            if (exit_ctr[0] % ACT_EXIT_DEN) < ACT_EXIT_NUM:
                                nc.scalar.activation(
                                    sq[:, cs], ps_s, AF.Square,
                                    bias=bias_zero, scale=0.25)
                            else:
                                e_s = im_p.tile([P, CH], f16, tag="es")
                                nc.vector.tensor_scalar_mul(e_s, ps_s, 0.25)
                                nc.vector.tensor_tensor(
                                    sq[:, cs], e_s, e_s, ALU.mult)
                            exit_ctr[0] += 1
                        if b4 % 2 == 1:  # QUAD-wide adds as halves complete
                            qs = slice((b4 - 1) * CH, (b4 + 1) * CH)
                            nc.gpsimd.tensor_tensor(
                                s_t[:, qs], sq1[:, qs], sq2[:, qs], ALU.add)
                    return s_t

                def stage_b(blocks, p_t, first, last,
                            ps_out=ps_out, ps_sums=ps_sums, j=j):
                    """causal mask + row-sum + PV accumulation matmuls.
                    Masking here (behind stage_a2) keeps the gpsimd FIFO
                    from head-of-line blocking the adds."""
                    for b4, i in enumerate(blocks):
                        p_blk = p_t[:, b4 * CH:(b4 + 1) * CH]
                        if i >= 4 * j:  # diagonal: zero where tq < tk
                            nc.gpsimd.affine_select(
                                out=p_blk, in_=p_blk,
                                compare_op=ALU.is_ge,
                                fill=0.0,
                                base=j * CH - i * P,
                                pattern=[[1, CH]],
                                channel_multiplier=-1)
                    for b4, i in enumerate(blocks):
                        p_blk = p_t[:, b4 * CH:(b4 + 1) * CH]
                        st = first and b4 == 0
                        sp = last and b4 == len(blocks) - 1
                        nc.tensor.matmul(ps_sums, ones_col, p_blk,
                                         start=st, stop=sp)
                        nc.tensor.matmul(ps_out, v_nat[:, i], p_blk,
                                         start=st, stop=sp)

                def finalize(j=j, ps_out=ps_out, ps_sums=ps_sums):
                    outT = fin_p.tile([P, CH], b16, tag="outT")
                    nc.vector.tensor_copy(outT, ps_out)
                    sums_sb = fin_p.tile([1, CH], f32, tag="sums")
                    nc.vector.tensor_copy(sums_sb, ps_sums)

                    ps_on = finps.tile([P, 4, H2], b16, tag="vno")
                    for t4 in range(4):
                        nc.tensor.transpose(
                            ps_on[:, t4], outT[:, t4 * P:(t4 + 1) * P], ident)
                    ps_rs = scmix.tile([P, CH], f32, tag="mix")
                    for t4 in range(4):
                        nc.tensor.matmul(ps_rs[:, t4:t4 + 1],
                                         sums_sb[0:1, t4 * P:(t4 + 1) * P],
                                         one1_f32, start=True, stop=True)
                    recip = fin_p.tile([P, 4], f32, tag="recip")
                    nc.vector.reciprocal(recip, ps_rs[:, 0:4])

                    onat = fin_p.tile([P, 4, H2], f32, tag="onat")
                    for t4 in range(4):
                        nc.vector.tensor_scalar_mul(
                            onat[:, t4], ps_on[:, t4], recip[:, t4:t4 + 1])
                    nc.scalar.dma_start(out_d[j], onat)

                # ~19 generator steps spread evenly over this chunk's groups
                steps = max(1, -(-19 // len(groups)))
                for gi, blocks in enumerate(groups):
                    s_t = stage_a1(blocks)
                    last = gi == len(groups) - 1
                    pend_a2.append(
                        (stage_b, blocks, s_t, gi == 0, last,
                         finalize if last else None))
                    flush(1, 1)
                    for _ in range(steps):
                        if gen is None or next(gen, "END") == "END":
                            gen = None
                            break
                if gen is not None:
                    for _ in gen:
                        pass

            # drain the last chunk (its finalize rides the last B entry)
            flush(0, 0)

    _split_multiwaits(nc)
    return nc


def _split_multiwaits(nc):
    """This toolchain's walrus accepts at most ONE sync-wait per instruction;
    Tile's sem-assignment can attach several. Hoist all-but-one wait onto
    standalone InstEventSemaphore carriers (what bass's wait_ge emits)."""
    import concourse.mybir as mybir

    n_split = 0
    for f in nc.m.functions:
        for bb in f.blocks:
            out = []
            for inst in bb.instructions:
                si = inst.sync_info
                if si is not None and si.on_wait and len(si.on_wait) > 1:
                    waits = list(si.on_wait)
                    for w in waits[:-1]:
                        carrier = mybir.InstEventSemaphore(
                            name=f"{inst.name}_wsplit{n_split}", ins=[], outs=[])
                        carrier.engine = inst.engine
                        carrier.sync_info = mybir.SyncInfo(
                            on_wait=[w], on_update=[])
                        out.append(carrier)
                        n_split += 1
                    inst.sync_info = mybir.SyncInfo(
                        on_wait=[waits[-1]], on_update=list(si.on_update))
                out.append(inst)
            bb.instructions = out
    return n_split


def _host_prep(Wk_r, Wk_i, Wq_r, Wq_i, Wv_r, Wv_i):
    import ml_dtypes

    f16 = np.float16
    b16 = ml_dtypes.bfloat16

    def s1(wr, wi):
        return np.concatenate([wr, wi], axis=1)

    def s2(wr, wi):
        return np.concatenate([-wi, wr], axis=1)

    # [pair(3), 2, C, H2] -> partition-major [P, 3, 2, CC, H2]
    wst = np.stack([
        [s1(Wk_r, Wk_i), s2(Wk_r, Wk_i)],
        [s1(Wq_r, Wq_i), s2(Wq_r, Wq_i)],
        [s1(Wv_r, Wv_i), s2(Wv_r, Wv_i)],
    ]).astype(f16)
    wst = np.ascontiguousarray(
        wst.reshape(3, 2, C // P, P, H2).transpose(3, 0, 1, 2, 4))
    # S with S @ [qr; qi] = [-qi; qr]; matmul computes lhsT.T @ rhs so pass S^T
    s_mat = np.zeros((P, P), np.float32)
    for i in range(H):
        s_mat[i, H + i] = -1.0
        s_mat[H + i, i] = 1.0
    cper = np.ascontiguousarray(s_mat.T).astype(f16)
    cidn = np.ascontiguousarray(np.concatenate(
        [np.eye(P, dtype=np.float32), np.ones((P, 1), np.float32)],
        axis=1)).astype(b16)
    return wst, cper, cidn


def kernel(x_real, x_imag, Wk_r, Wk_i, Wq_r, Wq_i, Wv_r, Wv_i, _trace=False):
    global _BUILT
    from concourse.bass_utils import run_bass_kernel_spmd

    if _BUILT is None:
        _BUILT = _build()
    nc = _BUILT

    wst, cper, cidn = _host_prep(
        np.asarray(Wk_r), np.asarray(Wk_i), np.asarray(Wq_r),
        np.asarray(Wq_i), np.asarray(Wv_r), np.asarray(Wv_i))
    x_real = np.asarray(x_real, dtype=np.float32)
    x_imag = np.asarray(x_imag, dtype=np.float32)

    def xprep(xb):
        # (T, C) -> xT (C, T) -> [chunk, part, p, 2, t] partition-major, fp16
        return np.ascontiguousarray(
            xb.T.reshape(4, 2, P, NCHUNK, CH).transpose(3, 0, 2, 1, 4)
            .astype(np.float16))

    in_maps = [
        {
            "xT_real": xprep(x_real[b]),
            "xT_imag": xprep(x_imag[b]),
            "wstacks": wst,
            "consts_perm": cper,
            "consts_ident": cidn,
        }
        for b in range(B)
    ]
    res = run_bass_kernel_spmd(nc, in_maps, core_ids=list(range(B)),
                               trace=_trace)
    def unpack(pk):
        # [chunk, p, tb, h2] -> (T, H2)
        full = pk.transpose(0, 2, 1, 3).reshape(T, H2)
        return full[:, 0:H], full[:, H:H2]

    outs = [unpack(res.results[b]["out_pk"]) for b in range(B)]
    out_r = np.ascontiguousarray(np.stack([o[0] for o in outs]))
    out_i = np.ascontiguousarray(np.stack([o[1] for o in outs]))
    if _trace:
        kernel._last_results = res
    return out_r, out_i
